# revision 7
# baseline (speedup 1.0000x reference)
"""Child-Sum TreeLSTM (complete binary trees) on 8 TRN2 NeuronCores.

Problem: B=256 trees, N=511 nodes (depth 9), D_IN=300, H=512.
Sharding: data-parallel over trees -- 32 trees per core, weights replicated.

Device algorithm (per core, all 32 trees):
  One uniform bottom-up level loop (level 8 = leaves ... level 0 = root).
  Everything is kept feature-on-partition ("transposed") so no on-device
  transposes are needed:
    x^T   [300pad384, cols]  (host pre-transposed, level-major column order)
    h^T/c^T per level [512=4x128, cols]  col = tree*2^l + node
  With tree-major column order, the children of parent column j are child
  columns 2j, 2j+1 of the level below -- child-sum and f*c reductions are
  stride-2 vector ops, and each parent chunk [p0,p0+P) consumes exactly
  child columns [2p0, 2p0+2P).

  Per level, per 512-column chunk:
    iou^T accumulated in PSUM: W_ioux^T x^T (3 K-chunks) + W_iouh^T hsum^T
    (4 K-chunks), evacuated by ACT with fused bias+sigmoid/tanh.
    f^T for both children in one go over child columns: W_fh^T h_child^T
    + W_fx^T x_dup^T (parent x duplicated via 0-step AP), sigmoid evac.
    c = sig(i)*tanh(u) + f_even*c_even + f_odd*c_odd ; h = sig(o)*tanh(c).
  h,c level state is bounced through DRAM scratch (SBUF can't hold the
  leaf levels); chunk-granular dependencies let Tile pipeline levels.

  Matmuls run as float32r (full-rate fp32, ~1e-4 rel err on TRN2).
"""

import sys

sys.path.insert(0, "/opt/trn_rl_repo")

from contextlib import ExitStack

import numpy as np

import concourse.bass as bass
import concourse.tile as tile
from concourse import bacc, mybir

F32 = mybir.dt.float32
F32R = mybir.dt.float32r
AFT = mybir.ActivationFunctionType

B, NTREE, DIN, H = 256, 511, 300, 512
NCORES = 8
BC = B // NCORES  # 32 trees per core
DEPTH = 9
KX = 3  # K chunks for D_IN (300 -> 3*128 padded)
KH = 4  # K chunks for H (512 = 4*128)
LCOLS = [BC * (1 << l) for l in range(DEPTH)]  # cols per level, index=level
TOTCOLS = sum(LCOLS)  # 16352
# column offset of each level in x^T (level-major, descending level)
LOFF = {}
_off = 0
for _l in range(DEPTH - 1, -1, -1):
    LOFF[_l] = _off
    _off += LCOLS[_l]

CHUNK = 512
# chunk-major layout tables: per level (desc), chunk width + number of chunks
PW = {l: min(CHUNK, LCOLS[l]) for l in range(DEPTH)}
NCH = {l: (LCOLS[l] + PW[l] - 1) // PW[l] for l in range(DEPTH)}
# global chunk index base per level for the x^T slab
CIBASE = {}
_ci = 0
for _l in range(DEPTH - 1, -1, -1):
    CIBASE[_l] = _ci
    _ci += NCH[_l]
NCHTOT = _ci  # 35


def build_program():
    nc = bacc.Bacc("TRN2", target_bir_lowering=False, debug=False)

    d_xt = nc.dram_tensor("xt", [NCHTOT, 128, KX * CHUNK], F32, kind="ExternalInput").ap()
    d_wioux = nc.dram_tensor("wioux", [KX, 128, 3 * H], F32, kind="ExternalInput").ap()
    d_wiouh = nc.dram_tensor("wiouh", [KH, 128, 3 * H], F32, kind="ExternalInput").ap()
    d_wfx = nc.dram_tensor("wfx", [KX, 128, H], F32, kind="ExternalInput").ap()
    d_wfh = nc.dram_tensor("wfh", [KH, 128, H], F32, kind="ExternalInput").ap()
    d_biou = nc.dram_tensor("biou", [128, 12], F32, kind="ExternalInput").ap()
    d_bf = nc.dram_tensor("bf", [128, 4], F32, kind="ExternalInput").ap()

    d_cout = nc.dram_tensor("c_out", [128, 4, BC], F32, kind="ExternalOutput").ap()
    d_hout = nc.dram_tensor("h_out", [128, 4, BC], F32, kind="ExternalOutput").ap()

    with tile.TileContext(nc) as tc, ExitStack() as ctx:
        wpool = ctx.enter_context(tc.tile_pool(name="weights", bufs=1))
        xpool = ctx.enter_context(tc.tile_pool(name="x", bufs=3))
        hchp = ctx.enter_context(tc.tile_pool(name="hch", bufs=2))
        cchp = ctx.enter_context(tc.tile_pool(name="cch", bufs=2))
        outp = ctx.enter_context(tc.tile_pool(name="state", bufs=2))
        workp = ctx.enter_context(tc.tile_pool(name="work", bufs=3))
        fcp = ctx.enter_context(tc.tile_pool(name="fc", bufs=2))
        hsump = ctx.enter_context(tc.tile_pool(name="hsum", bufs=2))
        psump = ctx.enter_context(tc.tile_pool(name="psum", bufs=8, space="PSUM"))
        dramp = ctx.enter_context(tc.tile_pool(name="dram", bufs=1, space="DRAM"))

        # ---- weights / biases (one-time casting DMAs to f32r) ----
        s_wioux = wpool.tile([128, KX, 3 * H], F32R)
        s_wiouh = wpool.tile([128, KH, 3 * H], F32R)
        s_wfx = wpool.tile([128, KX, H], F32R)
        s_wfh = wpool.tile([128, KH, H], F32R)
        for k in range(KX):
            nc.gpsimd.dma_start(out=s_wioux[:, k, :], in_=d_wioux[k])
            nc.gpsimd.dma_start(out=s_wfx[:, k, :], in_=d_wfx[k])
        for k in range(KH):
            nc.gpsimd.dma_start(out=s_wiouh[:, k, :], in_=d_wiouh[k])
            nc.gpsimd.dma_start(out=s_wfh[:, k, :], in_=d_wfh[k])
        s_biou = wpool.tile([128, 12], F32)
        s_bf = wpool.tile([128, 4], F32)
        nc.sync.dma_start(out=s_biou, in_=d_biou)
        nc.sync.dma_start(out=s_bf, in_=d_bf)

        # ---- DRAM scratch for per-level h/c state (levels 8..1) ----
        hd = {}
        cd = {}
        for l in range(1, DEPTH):
            if LCOLS[l] <= CHUNK:
                continue  # single-chunk levels stay SBUF-resident
            hd[l] = dramp.tile([NCH[l], 128, KH, PW[l]], F32R, tag=f"hd{l}", name=f"hd{l}")
            cd[l] = dramp.tile([NCH[l], 128, KH, PW[l]], F32, tag=f"cd{l}", name=f"cd{l}")

        def dup_ap(base):
            """Each column of `base` twice: [128, W] -> [128, W, 2] (0-step)."""
            return bass.AP(
                tensor=base.tensor,
                offset=base.offset,
                ap=list(base.ap) + [[0, 2]],
            )

        def iou_psum(m, P, xt, hsum):
            """PSUM accumulation for iou feature chunk m over P cols."""
            ps = psump.tile([128, CHUNK], F32, tag="ps")
            last_x = hsum is None
            for k in range(KX):
                nc.tensor.matmul(
                    ps[:, :P],
                    s_wioux[:, k, 128 * m : 128 * m + 128],
                    xt[:, k, :P],
                    start=(k == 0),
                    stop=(last_x and k == KX - 1),
                )
            if hsum is not None:
                for k in range(KH):
                    nc.tensor.matmul(
                        ps[:, :P],
                        s_wiouh[:, k, 128 * m : 128 * m + 128],
                        hsum[:, k, :P],
                        start=False,
                        stop=(k == KH - 1),
                    )
            return ps

        sbuf_child = {}  # level -> (ht, ct, width) for single-chunk levels

        def process_level(l):
            C = LCOLS[l]
            P = min(CHUNK, C)
            is_leaf = l == DEPTH - 1
            for j in range(0, C, P):
                # x^T chunk [128, KX, 512] -- one contiguous chunk-major load
                ci = CIBASE[l] + j // P
                xt = xpool.tile([128, KX, CHUNK], F32R, tag="xt")
                nc.gpsimd.dma_start(
                    out=xt[:].rearrange("p k c -> p (k c)"), in_=d_xt[ci]
                )

                ct = outp.tile([128, KH, CHUNK], F32, tag="ct")
                ht = outp.tile([128, KH, CHUNK], F32R, tag="ht")

                hsum = None
                if not is_leaf:
                    # children: cols [2j, 2j+2P) of level l+1 = child chunks
                    if l + 1 in sbuf_child:
                        hc0, cc0, pw = sbuf_child[l + 1]
                        npieces = 1
                        hch, cch = [hc0], [cc0]
                    else:
                        pw = PW[l + 1]
                        npieces = 2 * P // pw
                        cj0 = 2 * j // pw
                        hch, cch = [], []
                        for pc in range(npieces):
                            hc = hchp.tile([128, KH, CHUNK], F32R, tag="hch")
                            cc = cchp.tile([128, KH, CHUNK], F32, tag="cch")
                            for k in range(KH):
                                nc.sync.dma_start(
                                    out=hc[:, k, :pw], in_=hd[l + 1][cj0 + pc, :, k, :]
                                )
                                nc.sync.dma_start(
                                    out=cc[:, k, :pw], in_=cd[l + 1][cj0 + pc, :, k, :]
                                )
                            hch.append(hc)
                            cch.append(cc)

                    # hsum[:, :, a:a+pw/2] = hch[...,0::2] + [...,1::2]
                    hsum = hsump.tile([128, KH, CHUNK], F32R, tag="hsum")
                    for pc in range(npieces):
                        pair = hch[pc][:, :, :pw].rearrange(
                            "p k (n two) -> p k n two", two=2
                        )
                        a = pc * (pw // 2)
                        nc.gpsimd.tensor_add(
                            out=hsum[:, :, a : a + pw // 2],
                            in0=pair[:, :, :, 0],
                            in1=pair[:, :, :, 1],
                        )

                # ---- i/u gates: c = sigmoid(i) * tanh(u) ----
                for m in range(4):
                    ps_u = iou_psum(8 + m, P, xt, hsum)
                    tu = workp.tile([128, CHUNK], F32, tag="tu")
                    nc.scalar.activation(
                        tu[:, :P], ps_u[:, :P], AFT.Tanh, bias=s_biou[:, 8 + m : 9 + m]
                    )
                    ps_i = iou_psum(m, P, xt, hsum)
                    nc.scalar.activation(
                        ct[:, m, :P], ps_i[:, :P], AFT.Sigmoid, bias=s_biou[:, m : m + 1]
                    )
                    nc.vector.tensor_mul(ct[:, m, :P], ct[:, m, :P], tu[:, :P])

                # ---- forget gates + fc accumulation into c ----
                if not is_leaf:
                    for m in range(4):
                        fc = fcp.tile([128, 2 * CHUNK], F32, tag="fc")
                        for pc in range(npieces):
                            s = pc * pw
                            ps = psump.tile([128, CHUNK], F32, tag="ps")
                            for k in range(KH):
                                nc.tensor.matmul(
                                    ps[:, :pw],
                                    s_wfh[:, k, 128 * m : 128 * m + 128],
                                    hch[pc][:, k, :pw],
                                    start=(k == 0),
                                    stop=False,
                                )
                            # fx part: parent x cols duplicated twice (0-step AP)
                            for k in range(KX):
                                xb = xt[:, k, s // 2 : s // 2 + pw // 2]
                                nc.tensor.matmul(
                                    ps[:, :pw],
                                    s_wfx[:, k, 128 * m : 128 * m + 128],
                                    dup_ap(xb),
                                    start=False,
                                    stop=(k == KX - 1),
                                )
                            f = workp.tile([128, CHUNK], F32, tag="f")
                            nc.scalar.activation(
                                f[:, :pw], ps[:, :pw], AFT.Sigmoid, bias=s_bf[:, m : m + 1]
                            )
                            nc.vector.tensor_mul(
                                fc[:, s : s + pw], f[:, :pw], cch[pc][:, m, :pw]
                            )
                        fpair = fc[:, : 2 * P].rearrange("p (n two) -> p n two", two=2)
                        nc.gpsimd.tensor_add(
                            out=ct[:, m, :P], in0=ct[:, m, :P], in1=fpair[:, :, 0]
                        )
                        nc.gpsimd.tensor_add(
                            out=ct[:, m, :P], in0=ct[:, m, :P], in1=fpair[:, :, 1]
                        )

                # ---- o gates, h = sigmoid(o) * tanh(c) ----
                for m in range(4):
                    ps_o = iou_psum(4 + m, P, xt, hsum)
                    so = workp.tile([128, CHUNK], F32, tag="so")
                    nc.scalar.activation(
                        so[:, :P], ps_o[:, :P], AFT.Sigmoid, bias=s_biou[:, 4 + m : 5 + m]
                    )
                    tc_ = workp.tile([128, CHUNK], F32, tag="tc")
                    nc.scalar.activation(tc_[:, :P], ct[:, m, :P], AFT.Tanh)
                    nc.vector.tensor_mul(ht[:, m, :P], so[:, :P], tc_[:, :P])

                # ---- store state ----
                if l == 0:
                    nc.sync.dma_start(out=d_hout, in_=ht[:, :, :P].bitcast(F32))
                    nc.sync.dma_start(out=d_cout, in_=ct[:, :, :P])
                elif C <= CHUNK:
                    # single-chunk level: children consumed straight from SBUF
                    sbuf_child[l] = (ht, ct, P)
                else:
                    for k in range(KH):
                        nc.sync.dma_start(out=hd[l][j // P, :, k, :], in_=ht[:, k, :P])
                        nc.sync.dma_start(out=cd[l][j // P, :, k, :], in_=ct[:, k, :P])

        for l in range(DEPTH - 1, -1, -1):
            process_level(l)

    nc.compile()
    return nc


_nc_cache = None


def get_program():
    global _nc_cache
    if _nc_cache is None:
        _nc_cache = build_program()
    return _nc_cache


def prep_inputs(inputs, W_ioux, b_ioux, W_iouh, b_iouh, W_fx, b_fx, W_fh, b_fh):
    """Host-side prep: per-core x^T slabs + padded weight chunks + fused biases."""
    inputs = np.ascontiguousarray(np.asarray(inputs, dtype=np.float32))

    def padk(w, rows):
        w = np.asarray(w, np.float32)
        out = np.zeros((rows * 128, w.shape[1]), np.float32)
        out[: w.shape[0]] = w
        return np.ascontiguousarray(out.reshape(rows, 128, w.shape[1]))

    wioux = padk(W_ioux, KX)
    wiouh = padk(W_iouh, KH)
    wfx = padk(W_fx, KX)
    wfh = padk(W_fh, KH)
    biou = np.ascontiguousarray(
        (np.asarray(b_ioux) + np.asarray(b_iouh)).astype(np.float32).reshape(12, 128).T
    )
    bf = np.ascontiguousarray(
        (np.asarray(b_fx) + np.asarray(b_fh)).astype(np.float32).reshape(4, 128).T
    )

    in_maps = []
    for c in range(NCORES):
        xc = inputs[c * BC : (c + 1) * BC]  # [BC, NTREE, DIN]
        blocks = []
        for l in range(DEPTH - 1, -1, -1):
            a, b = (1 << l) - 1, (1 << (l + 1)) - 1
            blocks.append(xc[:, a:b, :].reshape(-1, DIN))  # tree-major
        xcols = np.concatenate(blocks, axis=0)  # [TOTCOLS, DIN]
        xtf = np.zeros((KX * 128, TOTCOLS), np.float32)
        xtf[:DIN] = xcols.T
        xtf = xtf.reshape(KX, 128, TOTCOLS)
        xt = np.zeros((NCHTOT, 128, KX, CHUNK), np.float32)
        for l in range(DEPTH - 1, -1, -1):
            for jj in range(NCH[l]):
                a = LOFF[l] + jj * PW[l]
                xt[CIBASE[l] + jj, :, :, : PW[l]] = xtf[:, :, a : a + PW[l]].transpose(
                    1, 0, 2
                )
        xt = np.ascontiguousarray(xt.reshape(NCHTOT, 128, KX * CHUNK))
        in_maps.append(
            {
                "xt": xt,
                "wioux": wioux,
                "wiouh": wiouh,
                "wfx": wfx,
                "wfh": wfh,
                "biou": biou,
                "bf": bf,
            }
        )
    return in_maps


def assemble_output(results):
    """results: list of per-core dicts with c_out/h_out [128, 4, BC]."""
    cs, hs = [], []
    for r in results:
        # [128part=feat%128, m=feat//128, tree] -> [tree, 512]
        c = np.transpose(r["c_out"], (2, 1, 0)).reshape(BC, H)
        h = np.transpose(r["h_out"], (2, 1, 0)).reshape(BC, H)
        cs.append(c)
        hs.append(h)
    return np.concatenate(cs, 0), np.concatenate(hs, 0)


def run_on_hw(in_maps, trace=False, tmpdir=None):
    from concourse.bass_utils import run_bass_kernel_spmd

    nc = get_program()
    return run_bass_kernel_spmd(
        nc, in_maps, list(range(NCORES)), trace=trace, tmpdir=tmpdir
    )


def kernel(**inputs):
    in_maps = prep_inputs(**inputs)
    res = run_on_hw(in_maps)
    return assemble_output(res.results)


# revision 8
# speedup vs baseline: 1.3266x; 1.3266x over previous
"""Child-Sum TreeLSTM (complete binary trees) on 8 TRN2 NeuronCores.

Problem: B=256 trees, N=511 nodes (depth 9), D_IN=300, H=512.
Sharding: data-parallel over trees -- 32 trees per core, weights replicated.

Device algorithm (per core, all 32 trees):
  One uniform bottom-up level loop (level 8 = leaves ... level 0 = root).
  Everything is kept feature-on-partition ("transposed") so no on-device
  transposes are needed:
    x^T   [300pad384, cols]  (host pre-transposed, level-major column order)
    h^T/c^T per level [512=4x128, cols]  col = tree*2^l + node
  With tree-major column order, the children of parent column j are child
  columns 2j, 2j+1 of the level below -- child-sum and f*c reductions are
  stride-2 vector ops, and each parent chunk [p0,p0+P) consumes exactly
  child columns [2p0, 2p0+2P).

  Per level, per 512-column chunk:
    iou^T accumulated in PSUM: W_ioux^T x^T (3 K-chunks) + W_iouh^T hsum^T
    (4 K-chunks), evacuated by ACT with fused bias+sigmoid/tanh.
    f^T for both children in one go over child columns: W_fh^T h_child^T
    + W_fx^T x_dup^T (parent x duplicated via 0-step AP), sigmoid evac.
    c = sig(i)*tanh(u) + f_even*c_even + f_odd*c_odd ; h = sig(o)*tanh(c).
  h,c level state is bounced through DRAM scratch (SBUF can't hold the
  leaf levels); chunk-granular dependencies let Tile pipeline levels.

  Matmuls run as float32r (full-rate fp32, ~1e-4 rel err on TRN2).
"""

import sys

sys.path.insert(0, "/opt/trn_rl_repo")

from contextlib import ExitStack

import numpy as np

import concourse.bass as bass
import concourse.tile as tile
from concourse import bacc, mybir

F32 = mybir.dt.float32
F32R = mybir.dt.float32r
BF16 = mybir.dt.bfloat16
AFT = mybir.ActivationFunctionType

B, NTREE, DIN, H = 256, 511, 300, 512
NCORES = 8
BC = B // NCORES  # 32 trees per core
DEPTH = 9
KX = 3  # K chunks for D_IN (300 -> 3*128 padded)
KH = 4  # K chunks for H (512 = 4*128)
LCOLS = [BC * (1 << l) for l in range(DEPTH)]  # cols per level, index=level
TOTCOLS = sum(LCOLS)  # 16352
# column offset of each level in x^T (level-major, descending level)
LOFF = {}
_off = 0
for _l in range(DEPTH - 1, -1, -1):
    LOFF[_l] = _off
    _off += LCOLS[_l]

CHUNK = 512
# chunk-major layout tables: per level (desc), chunk width + number of chunks
PW = {l: min(CHUNK, LCOLS[l]) for l in range(DEPTH)}
NCH = {l: (LCOLS[l] + PW[l] - 1) // PW[l] for l in range(DEPTH)}
# global chunk index base per level for the x^T slab
CIBASE = {}
_ci = 0
for _l in range(DEPTH - 1, -1, -1):
    CIBASE[_l] = _ci
    _ci += NCH[_l]
NCHTOT = _ci  # 35


def build_program():
    nc = bacc.Bacc("TRN2", target_bir_lowering=False, debug=False)

    d_xt = nc.dram_tensor("xt", [NCHTOT, 128, KX * CHUNK], BF16, kind="ExternalInput").ap()
    d_wioux = nc.dram_tensor("wioux", [KX, 128, 3 * H], BF16, kind="ExternalInput").ap()
    d_wiouh = nc.dram_tensor("wiouh", [KH, 128, 3 * H], BF16, kind="ExternalInput").ap()
    d_wfx = nc.dram_tensor("wfx", [KX, 128, H], BF16, kind="ExternalInput").ap()
    d_wfh = nc.dram_tensor("wfh", [KH, 128, H], BF16, kind="ExternalInput").ap()
    d_biou = nc.dram_tensor("biou", [128, 12], F32, kind="ExternalInput").ap()
    d_bf = nc.dram_tensor("bf", [128, 4], F32, kind="ExternalInput").ap()

    d_cout = nc.dram_tensor("c_out", [128, 4, BC], F32, kind="ExternalOutput").ap()
    d_hout = nc.dram_tensor("h_out", [128, 4, BC], F32, kind="ExternalOutput").ap()

    with tile.TileContext(nc) as tc, ExitStack() as ctx:
        wpool = ctx.enter_context(tc.tile_pool(name="weights", bufs=1))
        xpool = ctx.enter_context(tc.tile_pool(name="x", bufs=3))
        hchp = ctx.enter_context(tc.tile_pool(name="hch", bufs=2))
        cchp = ctx.enter_context(tc.tile_pool(name="cch", bufs=2))
        outp = ctx.enter_context(tc.tile_pool(name="state", bufs=2))
        workp = ctx.enter_context(tc.tile_pool(name="work", bufs=3))
        fcp = ctx.enter_context(tc.tile_pool(name="fc", bufs=2))
        hsump = ctx.enter_context(tc.tile_pool(name="hsum", bufs=2))
        psump = ctx.enter_context(tc.tile_pool(name="psum", bufs=8, space="PSUM"))
        dramp = ctx.enter_context(tc.tile_pool(name="dram", bufs=1, space="DRAM"))

        # ---- weights / biases (one-time casting DMAs to f32r) ----
        s_wioux = wpool.tile([128, KX, 3 * H], BF16)
        s_wiouh = wpool.tile([128, KH, 3 * H], BF16)
        s_wfx = wpool.tile([128, KX, H], BF16)
        s_wfh = wpool.tile([128, KH, H], BF16)
        for k in range(KX):
            nc.sync.dma_start(out=s_wioux[:, k, :], in_=d_wioux[k])
            nc.sync.dma_start(out=s_wfx[:, k, :], in_=d_wfx[k])
        for k in range(KH):
            nc.sync.dma_start(out=s_wiouh[:, k, :], in_=d_wiouh[k])
            nc.sync.dma_start(out=s_wfh[:, k, :], in_=d_wfh[k])
        s_biou = wpool.tile([128, 12], F32)
        s_bf = wpool.tile([128, 4], F32)
        nc.sync.dma_start(out=s_biou, in_=d_biou)
        nc.sync.dma_start(out=s_bf, in_=d_bf)

        # ---- DRAM scratch for per-level h/c state (levels 8..1) ----
        hd = {}
        cd = {}
        for l in range(1, DEPTH):
            if LCOLS[l] <= CHUNK:
                continue  # single-chunk levels stay SBUF-resident
            hd[l] = dramp.tile([NCH[l], 128, KH, PW[l]], BF16, tag=f"hd{l}", name=f"hd{l}")
            cd[l] = dramp.tile([NCH[l], 128, KH, PW[l]], F32, tag=f"cd{l}", name=f"cd{l}")

        def dup_ap(base):
            """Each column of `base` twice: [128, W] -> [128, W, 2] (0-step)."""
            return bass.AP(
                tensor=base.tensor,
                offset=base.offset,
                ap=list(base.ap) + [[0, 2]],
            )

        def iou_psum(m, P, xt, hsum):
            """PSUM accumulation for iou feature chunk m over P cols."""
            ps = psump.tile([128, CHUNK], F32, tag="ps")
            last_x = hsum is None
            for k in range(KX):
                nc.tensor.matmul(
                    ps[:, :P],
                    s_wioux[:, k, 128 * m : 128 * m + 128],
                    xt[:, k, :P],
                    start=(k == 0),
                    stop=(last_x and k == KX - 1),
                )
            if hsum is not None:
                for k in range(KH):
                    nc.tensor.matmul(
                        ps[:, :P],
                        s_wiouh[:, k, 128 * m : 128 * m + 128],
                        hsum[:, k, :P],
                        start=False,
                        stop=(k == KH - 1),
                    )
            return ps

        sbuf_child = {}  # level -> (ht, ct, width) for single-chunk levels

        def process_level(l):
            C = LCOLS[l]
            P = min(CHUNK, C)
            is_leaf = l == DEPTH - 1
            for j in range(0, C, P):
                # x^T chunk [128, KX, 512] -- one contiguous chunk-major load
                ci = CIBASE[l] + j // P
                xt = xpool.tile([128, KX, CHUNK], BF16, tag="xt")
                nc.sync.dma_start(
                    out=xt[:].rearrange("p k c -> p (k c)"), in_=d_xt[ci]
                )

                ct = outp.tile([128, KH, CHUNK], F32, tag="ct")
                ht = outp.tile([128, KH, CHUNK], BF16, tag="ht")

                hsum = None
                if not is_leaf:
                    # children: cols [2j, 2j+2P) of level l+1 = child chunks
                    if l + 1 in sbuf_child:
                        hc0, cc0, pw = sbuf_child[l + 1]
                        npieces = 1
                        hch, cch = [hc0], [cc0]
                    else:
                        pw = PW[l + 1]
                        npieces = 2 * P // pw
                        cj0 = 2 * j // pw
                        hch, cch = [], []
                        for pc in range(npieces):
                            hc = hchp.tile([128, KH, CHUNK], BF16, tag="hch")
                            cc = cchp.tile([128, KH, CHUNK], F32, tag="cch")
                            nc.sync.dma_start(out=hc[:, :, :pw], in_=hd[l + 1][cj0 + pc])
                            nc.sync.dma_start(out=cc[:, :, :pw], in_=cd[l + 1][cj0 + pc])
                            hch.append(hc)
                            cch.append(cc)

                    # hsum[:, :, a:a+pw/2] = hch[...,0::2] + [...,1::2]
                    hsum = hsump.tile([128, KH, CHUNK], BF16, tag="hsum")
                    for pc in range(npieces):
                        pair = hch[pc][:, :, :pw].rearrange(
                            "p k (n two) -> p k n two", two=2
                        )
                        a = pc * (pw // 2)
                        nc.gpsimd.tensor_add(
                            out=hsum[:, :, a : a + pw // 2],
                            in0=pair[:, :, :, 0],
                            in1=pair[:, :, :, 1],
                        )

                # ---- i/u gates: c = sigmoid(i) * tanh(u) ----
                for m in range(4):
                    ps_u = iou_psum(8 + m, P, xt, hsum)
                    tu = workp.tile([128, CHUNK], F32, tag="tu")
                    nc.scalar.activation(
                        tu[:, :P], ps_u[:, :P], AFT.Tanh, bias=s_biou[:, 8 + m : 9 + m]
                    )
                    ps_i = iou_psum(m, P, xt, hsum)
                    nc.scalar.activation(
                        ct[:, m, :P], ps_i[:, :P], AFT.Sigmoid, bias=s_biou[:, m : m + 1]
                    )
                    nc.vector.tensor_mul(ct[:, m, :P], ct[:, m, :P], tu[:, :P])

                # ---- forget gates + fc accumulation into c ----
                if not is_leaf:
                    for m in range(4):
                        fc = fcp.tile([128, 2 * CHUNK], F32, tag="fc")
                        for pc in range(npieces):
                            s = pc * pw
                            ps = psump.tile([128, CHUNK], F32, tag="ps")
                            for k in range(KH):
                                nc.tensor.matmul(
                                    ps[:, :pw],
                                    s_wfh[:, k, 128 * m : 128 * m + 128],
                                    hch[pc][:, k, :pw],
                                    start=(k == 0),
                                    stop=False,
                                )
                            # fx part: parent x cols duplicated twice (0-step AP)
                            for k in range(KX):
                                xb = xt[:, k, s // 2 : s // 2 + pw // 2]
                                nc.tensor.matmul(
                                    ps[:, :pw],
                                    s_wfx[:, k, 128 * m : 128 * m + 128],
                                    dup_ap(xb),
                                    start=False,
                                    stop=(k == KX - 1),
                                )
                            f = workp.tile([128, CHUNK], F32, tag="f")
                            nc.scalar.activation(
                                f[:, :pw], ps[:, :pw], AFT.Sigmoid, bias=s_bf[:, m : m + 1]
                            )
                            nc.vector.tensor_mul(
                                fc[:, s : s + pw], f[:, :pw], cch[pc][:, m, :pw]
                            )
                        fpair = fc[:, : 2 * P].rearrange("p (n two) -> p n two", two=2)
                        nc.gpsimd.tensor_add(
                            out=ct[:, m, :P], in0=ct[:, m, :P], in1=fpair[:, :, 0]
                        )
                        nc.gpsimd.tensor_add(
                            out=ct[:, m, :P], in0=ct[:, m, :P], in1=fpair[:, :, 1]
                        )

                # ---- o gates, h = sigmoid(o) * tanh(c) ----
                for m in range(4):
                    ps_o = iou_psum(4 + m, P, xt, hsum)
                    so = workp.tile([128, CHUNK], F32, tag="so")
                    nc.scalar.activation(
                        so[:, :P], ps_o[:, :P], AFT.Sigmoid, bias=s_biou[:, 4 + m : 5 + m]
                    )
                    tc_ = workp.tile([128, CHUNK], F32, tag="tc")
                    nc.scalar.activation(tc_[:, :P], ct[:, m, :P], AFT.Tanh)
                    nc.vector.tensor_mul(ht[:, m, :P], so[:, :P], tc_[:, :P])

                # ---- store state ----
                if l == 0:
                    nc.gpsimd.dma_start(out=d_hout, in_=ht[:, :, :P])
                    nc.sync.dma_start(out=d_cout, in_=ct[:, :, :P])
                elif C <= CHUNK:
                    # single-chunk level: children consumed straight from SBUF
                    sbuf_child[l] = (ht, ct, P)
                else:
                    nc.sync.dma_start(out=hd[l][j // P], in_=ht[:, :, :P])
                    nc.sync.dma_start(out=cd[l][j // P], in_=ct[:, :, :P])

        for l in range(DEPTH - 1, -1, -1):
            process_level(l)

    nc.compile()
    return nc


_nc_cache = None


def get_program():
    global _nc_cache
    if _nc_cache is None:
        _nc_cache = build_program()
    return _nc_cache


def prep_inputs(inputs, W_ioux, b_ioux, W_iouh, b_iouh, W_fx, b_fx, W_fh, b_fh):
    """Host-side prep: per-core x^T slabs + padded weight chunks + fused biases."""
    inputs = np.ascontiguousarray(np.asarray(inputs, dtype=np.float32))

    import ml_dtypes

    BF = ml_dtypes.bfloat16

    def padk(w, rows):
        w = np.asarray(w, np.float32)
        out = np.zeros((rows * 128, w.shape[1]), np.float32)
        out[: w.shape[0]] = w
        return np.ascontiguousarray(out.reshape(rows, 128, w.shape[1]).astype(BF))

    wioux = padk(W_ioux, KX)
    wiouh = padk(W_iouh, KH)
    wfx = padk(W_fx, KX)
    wfh = padk(W_fh, KH)
    biou = np.ascontiguousarray(
        (np.asarray(b_ioux) + np.asarray(b_iouh)).astype(np.float32).reshape(12, 128).T
    )
    bf = np.ascontiguousarray(
        (np.asarray(b_fx) + np.asarray(b_fh)).astype(np.float32).reshape(4, 128).T
    )

    in_maps = []
    for c in range(NCORES):
        xc = inputs[c * BC : (c + 1) * BC]  # [BC, NTREE, DIN]
        blocks = []
        for l in range(DEPTH - 1, -1, -1):
            a, b = (1 << l) - 1, (1 << (l + 1)) - 1
            blocks.append(xc[:, a:b, :].reshape(-1, DIN))  # tree-major
        xcols = np.concatenate(blocks, axis=0)  # [TOTCOLS, DIN]
        xtf = np.zeros((KX * 128, TOTCOLS), np.float32)
        xtf[:DIN] = xcols.T
        xtf = xtf.reshape(KX, 128, TOTCOLS)
        xt = np.zeros((NCHTOT, 128, KX, CHUNK), np.float32)
        for l in range(DEPTH - 1, -1, -1):
            for jj in range(NCH[l]):
                a = LOFF[l] + jj * PW[l]
                xt[CIBASE[l] + jj, :, :, : PW[l]] = xtf[:, :, a : a + PW[l]].transpose(
                    1, 0, 2
                )
        xt = np.ascontiguousarray(xt.reshape(NCHTOT, 128, KX * CHUNK).astype(BF))
        in_maps.append(
            {
                "xt": xt,
                "wioux": wioux,
                "wiouh": wiouh,
                "wfx": wfx,
                "wfh": wfh,
                "biou": biou,
                "bf": bf,
            }
        )
    return in_maps


def assemble_output(results):
    """results: list of per-core dicts with c_out/h_out [128, 4, BC]."""
    cs, hs = [], []
    for r in results:
        # [128part=feat%128, m=feat//128, tree] -> [tree, 512]
        c = np.transpose(r["c_out"], (2, 1, 0)).reshape(BC, H)
        h = np.transpose(r["h_out"], (2, 1, 0)).reshape(BC, H)
        cs.append(c)
        hs.append(h)
    return np.concatenate(cs, 0), np.concatenate(hs, 0)


def run_on_hw(in_maps, trace=False, tmpdir=None):
    from concourse.bass_utils import run_bass_kernel_spmd

    nc = get_program()
    return run_bass_kernel_spmd(
        nc, in_maps, list(range(NCORES)), trace=trace, tmpdir=tmpdir
    )


def kernel(**inputs):
    in_maps = prep_inputs(**inputs)
    res = run_on_hw(in_maps)
    return assemble_output(res.results)


# revision 9
# speedup vs baseline: 1.5469x; 1.1661x over previous
"""Child-Sum TreeLSTM (complete binary trees) on 8 TRN2 NeuronCores.

Problem: B=256 trees, N=511 nodes (depth 9), D_IN=300, H=512.
Sharding: data-parallel over trees -- 32 trees per core, weights replicated.

Device algorithm (per core, all 32 trees):
  One uniform bottom-up level loop (level 8 = leaves ... level 0 = root).
  Everything is kept feature-on-partition ("transposed") so no on-device
  transposes are needed:
    x^T   [300pad384, cols]  (host pre-transposed, level-major column order)
    h^T/c^T per level [512=4x128, cols]  col = tree*2^l + node
  With tree-major column order, the children of parent column j are child
  columns 2j, 2j+1 of the level below -- child-sum and f*c reductions are
  stride-2 vector ops, and each parent chunk [p0,p0+P) consumes exactly
  child columns [2p0, 2p0+2P).

  Per level, per 512-column chunk:
    iou^T accumulated in PSUM: W_ioux^T x^T (3 K-chunks) + W_iouh^T hsum^T
    (4 K-chunks), evacuated by ACT with fused bias+sigmoid/tanh.
    f^T for both children in one go over child columns: W_fh^T h_child^T
    + W_fx^T x_dup^T (parent x duplicated via 0-step AP), sigmoid evac.
    c = sig(i)*tanh(u) + f_even*c_even + f_odd*c_odd ; h = sig(o)*tanh(c).
  h,c level state is bounced through DRAM scratch (SBUF can't hold the
  leaf levels); chunk-granular dependencies let Tile pipeline levels.

  Matmuls run as float32r (full-rate fp32, ~1e-4 rel err on TRN2).
"""

import sys

sys.path.insert(0, "/opt/trn_rl_repo")

from contextlib import ExitStack

import numpy as np

import concourse.bass as bass
import concourse.tile as tile
from concourse import bacc, mybir

F32 = mybir.dt.float32
F32R = mybir.dt.float32r
BF16 = mybir.dt.bfloat16
AFT = mybir.ActivationFunctionType

B, NTREE, DIN, H = 256, 511, 300, 512
NCORES = 8
BC = B // NCORES  # 32 trees per core
DEPTH = 9
KX = 3  # K chunks for D_IN (300 -> 3*128 padded)
KH = 4  # K chunks for H (512 = 4*128)
LCOLS = [BC * (1 << l) for l in range(DEPTH)]  # cols per level, index=level
TOTCOLS = sum(LCOLS)  # 16352
# column offset of each level in x^T (level-major, descending level)
LOFF = {}
_off = 0
for _l in range(DEPTH - 1, -1, -1):
    LOFF[_l] = _off
    _off += LCOLS[_l]

CHUNK = 512
# chunk-major layout tables: per level (desc), chunk width + number of chunks
PW = {l: min(CHUNK, LCOLS[l]) for l in range(DEPTH)}
NCH = {l: (LCOLS[l] + PW[l] - 1) // PW[l] for l in range(DEPTH)}
# global chunk index base per level for the x^T slab
CIBASE = {}
_ci = 0
for _l in range(DEPTH - 1, -1, -1):
    CIBASE[_l] = _ci
    _ci += NCH[_l]
NCHTOT = _ci  # 35


def build_program():
    nc = bacc.Bacc("TRN2", target_bir_lowering=False, debug=False)

    d_xt = nc.dram_tensor("xt", [NCHTOT, 128, KX * CHUNK], BF16, kind="ExternalInput").ap()
    d_wioux = nc.dram_tensor("wioux", [KX, 128, 3 * H], BF16, kind="ExternalInput").ap()
    d_wiouh = nc.dram_tensor("wiouh", [KH, 128, 3 * H], BF16, kind="ExternalInput").ap()
    d_wfx = nc.dram_tensor("wfx", [KX, 128, H], BF16, kind="ExternalInput").ap()
    d_wfh = nc.dram_tensor("wfh", [KH, 128, H], BF16, kind="ExternalInput").ap()
    d_biou = nc.dram_tensor("biou", [128, 12], F32, kind="ExternalInput").ap()
    d_bf = nc.dram_tensor("bf", [128, 4], F32, kind="ExternalInput").ap()

    d_cout = nc.dram_tensor("c_out", [128, 4, BC], F32, kind="ExternalOutput").ap()
    d_hout = nc.dram_tensor("h_out", [128, 4, BC], F32, kind="ExternalOutput").ap()

    with tile.TileContext(nc) as tc, ExitStack() as ctx:
        wpool = ctx.enter_context(tc.tile_pool(name="weights", bufs=1))
        xpool = ctx.enter_context(tc.tile_pool(name="x", bufs=4))
        hchp = ctx.enter_context(tc.tile_pool(name="hch", bufs=4))
        cchp = ctx.enter_context(tc.tile_pool(name="cch", bufs=4))
        outp = ctx.enter_context(tc.tile_pool(name="state", bufs=2))
        workp = ctx.enter_context(tc.tile_pool(name="work", bufs=3))
        fcp = ctx.enter_context(tc.tile_pool(name="fc", bufs=2))
        hsump = ctx.enter_context(tc.tile_pool(name="hsum", bufs=3))
        psump = ctx.enter_context(tc.tile_pool(name="psum", bufs=8, space="PSUM"))
        dramp = ctx.enter_context(tc.tile_pool(name="dram", bufs=1, space="DRAM"))

        # ---- weights / biases (one-time casting DMAs to f32r) ----
        s_wioux = wpool.tile([128, KX, 3 * H], BF16)
        s_wiouh = wpool.tile([128, KH, 3 * H], BF16)
        s_wfx = wpool.tile([128, KX, H], BF16)
        s_wfh = wpool.tile([128, KH, H], BF16)
        for k in range(KX):
            nc.sync.dma_start(out=s_wioux[:, k, :], in_=d_wioux[k])
            nc.sync.dma_start(out=s_wfx[:, k, :], in_=d_wfx[k])
        for k in range(KH):
            nc.sync.dma_start(out=s_wiouh[:, k, :], in_=d_wiouh[k])
            nc.sync.dma_start(out=s_wfh[:, k, :], in_=d_wfh[k])
        s_biou = wpool.tile([128, 12], F32)
        s_bf = wpool.tile([128, 4], F32)
        nc.sync.dma_start(out=s_biou, in_=d_biou)
        nc.sync.dma_start(out=s_bf, in_=d_bf)

        # ---- DRAM scratch for per-level h/c state (levels 8..1) ----
        hd = {}
        cd = {}
        for l in range(1, DEPTH):
            if LCOLS[l] <= CHUNK:
                continue  # single-chunk levels stay SBUF-resident
            hd[l] = dramp.tile([NCH[l], 128, KH, PW[l]], BF16, tag=f"hd{l}", name=f"hd{l}")
            cd[l] = dramp.tile([NCH[l], 128, KH, PW[l]], F32, tag=f"cd{l}", name=f"cd{l}")

        def dup_ap(base):
            """Each column of `base` twice: [128, W] -> [128, W, 2] (0-step)."""
            return bass.AP(
                tensor=base.tensor,
                offset=base.offset,
                ap=list(base.ap) + [[0, 2]],
            )

        def iou_psum(m, P, xt, hsum):
            """PSUM accumulation for iou feature chunk m over P cols."""
            ps = psump.tile([128, CHUNK], F32, tag="ps")
            last_x = hsum is None
            for k in range(KX):
                nc.tensor.matmul(
                    ps[:, :P],
                    s_wioux[:, k, 128 * m : 128 * m + 128],
                    xt[:, k, :P],
                    start=(k == 0),
                    stop=(last_x and k == KX - 1),
                )
            if hsum is not None:
                for k in range(KH):
                    nc.tensor.matmul(
                        ps[:, :P],
                        s_wiouh[:, k, 128 * m : 128 * m + 128],
                        hsum[:, k, :P],
                        start=False,
                        stop=(k == KH - 1),
                    )
            return ps

        sbuf_child = {}  # level -> (ht, ct, width) for single-chunk levels

        def process_level(l):
            C = LCOLS[l]
            P = min(CHUNK, C)
            is_leaf = l == DEPTH - 1
            for j in range(0, C, P):
                # x^T chunk [128, KX, 512] -- one contiguous chunk-major load
                ci = CIBASE[l] + j // P
                xt = xpool.tile([128, KX, CHUNK], BF16, tag="xt")
                nc.sync.dma_start(
                    out=xt[:].rearrange("p k c -> p (k c)"), in_=d_xt[ci]
                )

                ct = outp.tile([128, KH, CHUNK], F32, tag="ct")
                ht = outp.tile([128, KH, CHUNK], BF16, tag="ht")

                hsum = None
                if not is_leaf:
                    # children: cols [2j, 2j+2P) of level l+1 = child chunks
                    if l + 1 in sbuf_child:
                        hc0, cc0, pw = sbuf_child[l + 1]
                        npieces = 1
                        hch, cch = [hc0], [cc0]
                    else:
                        pw = PW[l + 1]
                        npieces = 2 * P // pw
                        cj0 = 2 * j // pw
                        hch, cch = [], []
                        for pc in range(npieces):
                            hc = hchp.tile([128, KH, CHUNK], BF16, tag="hch")
                            cc = cchp.tile([128, KH, CHUNK], F32, tag="cch")
                            nc.sync.dma_start(out=hc[:, :, :pw], in_=hd[l + 1][cj0 + pc])
                            nc.sync.dma_start(out=cc[:, :, :pw], in_=cd[l + 1][cj0 + pc])
                            hch.append(hc)
                            cch.append(cc)

                    # hsum[:, :, a:a+pw/2] = hch[...,0::2] + [...,1::2]
                    hsum = hsump.tile([128, KH, CHUNK], BF16, tag="hsum")
                    for pc in range(npieces):
                        pair = hch[pc][:, :, :pw].rearrange(
                            "p k (n two) -> p k n two", two=2
                        )
                        a = pc * (pw // 2)
                        nc.gpsimd.tensor_add(
                            out=hsum[:, :, a : a + pw // 2],
                            in0=pair[:, :, :, 0],
                            in1=pair[:, :, :, 1],
                        )

                # ---- i/u gates: c = sigmoid(i) * tanh(u) ----
                for m in range(4):
                    ps_u = iou_psum(8 + m, P, xt, hsum)
                    tu = workp.tile([128, CHUNK], F32, tag="tu")
                    nc.scalar.activation(
                        tu[:, :P], ps_u[:, :P], AFT.Tanh, bias=s_biou[:, 8 + m : 9 + m]
                    )
                    ps_i = iou_psum(m, P, xt, hsum)
                    nc.scalar.activation(
                        ct[:, m, :P], ps_i[:, :P], AFT.Sigmoid, bias=s_biou[:, m : m + 1]
                    )
                    nc.vector.tensor_mul(ct[:, m, :P], ct[:, m, :P], tu[:, :P])

                # ---- forget gates + fc accumulation into c ----
                if not is_leaf:
                    for m in range(4):
                        fc = fcp.tile([128, 2 * CHUNK], F32, tag="fc")
                        for pc in range(npieces):
                            s = pc * pw
                            ps = psump.tile([128, CHUNK], F32, tag="ps")
                            for k in range(KH):
                                nc.tensor.matmul(
                                    ps[:, :pw],
                                    s_wfh[:, k, 128 * m : 128 * m + 128],
                                    hch[pc][:, k, :pw],
                                    start=(k == 0),
                                    stop=False,
                                )
                            # fx part: parent x cols duplicated twice (0-step AP)
                            for k in range(KX):
                                xb = xt[:, k, s // 2 : s // 2 + pw // 2]
                                nc.tensor.matmul(
                                    ps[:, :pw],
                                    s_wfx[:, k, 128 * m : 128 * m + 128],
                                    dup_ap(xb),
                                    start=False,
                                    stop=(k == KX - 1),
                                )
                            f = workp.tile([128, CHUNK], F32, tag="f")
                            nc.scalar.activation(
                                f[:, :pw], ps[:, :pw], AFT.Sigmoid, bias=s_bf[:, m : m + 1]
                            )
                            nc.vector.tensor_mul(
                                fc[:, s : s + pw], f[:, :pw], cch[pc][:, m, :pw]
                            )
                        fpair = fc[:, : 2 * P].rearrange("p (n two) -> p n two", two=2)
                        nc.gpsimd.tensor_add(
                            out=ct[:, m, :P], in0=ct[:, m, :P], in1=fpair[:, :, 0]
                        )
                        nc.gpsimd.tensor_add(
                            out=ct[:, m, :P], in0=ct[:, m, :P], in1=fpair[:, :, 1]
                        )

                # ---- o gates, h = sigmoid(o) * tanh(c) ----
                for m in range(4):
                    ps_o = iou_psum(4 + m, P, xt, hsum)
                    so = workp.tile([128, CHUNK], F32, tag="so")
                    nc.scalar.activation(
                        so[:, :P], ps_o[:, :P], AFT.Sigmoid, bias=s_biou[:, 4 + m : 5 + m]
                    )
                    tc_ = workp.tile([128, CHUNK], F32, tag="tc")
                    nc.scalar.activation(tc_[:, :P], ct[:, m, :P], AFT.Tanh)
                    nc.vector.tensor_mul(ht[:, m, :P], so[:, :P], tc_[:, :P])

                # ---- store state ----
                if l == 0:
                    nc.gpsimd.dma_start(out=d_hout, in_=ht[:, :, :P])
                    nc.sync.dma_start(out=d_cout, in_=ct[:, :, :P])
                elif C <= CHUNK:
                    # single-chunk level: children consumed straight from SBUF
                    sbuf_child[l] = (ht, ct, P)
                else:
                    nc.sync.dma_start(out=hd[l][j // P], in_=ht[:, :, :P])
                    nc.sync.dma_start(out=cd[l][j // P], in_=ct[:, :, :P])

        for l in range(DEPTH - 1, -1, -1):
            process_level(l)

    nc.compile()
    return nc


_nc_cache = None


def get_program():
    global _nc_cache
    if _nc_cache is None:
        _nc_cache = build_program()
    return _nc_cache


def prep_inputs(inputs, W_ioux, b_ioux, W_iouh, b_iouh, W_fx, b_fx, W_fh, b_fh):
    """Host-side prep: per-core x^T slabs + padded weight chunks + fused biases."""
    inputs = np.ascontiguousarray(np.asarray(inputs, dtype=np.float32))

    import ml_dtypes

    BF = ml_dtypes.bfloat16

    def padk(w, rows):
        w = np.asarray(w, np.float32)
        out = np.zeros((rows * 128, w.shape[1]), np.float32)
        out[: w.shape[0]] = w
        return np.ascontiguousarray(out.reshape(rows, 128, w.shape[1]).astype(BF))

    wioux = padk(W_ioux, KX)
    wiouh = padk(W_iouh, KH)
    wfx = padk(W_fx, KX)
    wfh = padk(W_fh, KH)
    biou = np.ascontiguousarray(
        (np.asarray(b_ioux) + np.asarray(b_iouh)).astype(np.float32).reshape(12, 128).T
    )
    bf = np.ascontiguousarray(
        (np.asarray(b_fx) + np.asarray(b_fh)).astype(np.float32).reshape(4, 128).T
    )

    in_maps = []
    for c in range(NCORES):
        xc = inputs[c * BC : (c + 1) * BC]  # [BC, NTREE, DIN]
        blocks = []
        for l in range(DEPTH - 1, -1, -1):
            a, b = (1 << l) - 1, (1 << (l + 1)) - 1
            blocks.append(xc[:, a:b, :].reshape(-1, DIN))  # tree-major
        xcols = np.concatenate(blocks, axis=0)  # [TOTCOLS, DIN]
        xtf = np.zeros((KX * 128, TOTCOLS), np.float32)
        xtf[:DIN] = xcols.T
        xtf = xtf.reshape(KX, 128, TOTCOLS)
        xt = np.zeros((NCHTOT, 128, KX, CHUNK), np.float32)
        for l in range(DEPTH - 1, -1, -1):
            for jj in range(NCH[l]):
                a = LOFF[l] + jj * PW[l]
                xt[CIBASE[l] + jj, :, :, : PW[l]] = xtf[:, :, a : a + PW[l]].transpose(
                    1, 0, 2
                )
        xt = np.ascontiguousarray(xt.reshape(NCHTOT, 128, KX * CHUNK).astype(BF))
        in_maps.append(
            {
                "xt": xt,
                "wioux": wioux,
                "wiouh": wiouh,
                "wfx": wfx,
                "wfh": wfh,
                "biou": biou,
                "bf": bf,
            }
        )
    return in_maps


def assemble_output(results):
    """results: list of per-core dicts with c_out/h_out [128, 4, BC]."""
    cs, hs = [], []
    for r in results:
        # [128part=feat%128, m=feat//128, tree] -> [tree, 512]
        c = np.transpose(r["c_out"], (2, 1, 0)).reshape(BC, H)
        h = np.transpose(r["h_out"], (2, 1, 0)).reshape(BC, H)
        cs.append(c)
        hs.append(h)
    return np.concatenate(cs, 0), np.concatenate(hs, 0)


def run_on_hw(in_maps, trace=False, tmpdir=None):
    from concourse.bass_utils import run_bass_kernel_spmd

    nc = get_program()
    return run_bass_kernel_spmd(
        nc, in_maps, list(range(NCORES)), trace=trace, tmpdir=tmpdir
    )


def kernel(**inputs):
    in_maps = prep_inputs(**inputs)
    res = run_on_hw(in_maps)
    return assemble_output(res.results)


# revision 10
# speedup vs baseline: 1.6538x; 1.0691x over previous
"""Child-Sum TreeLSTM (complete binary trees) on 8 TRN2 NeuronCores.

Problem: B=256 trees, N=511 nodes (depth 9), D_IN=300, H=512.
Sharding: data-parallel over trees -- 32 trees per core, weights replicated.

Device algorithm (per core, all 32 trees):
  One uniform bottom-up level loop (level 8 = leaves ... level 0 = root).
  Everything is kept feature-on-partition ("transposed") so no on-device
  transposes are needed:
    x^T   [300pad384, cols]  (host pre-transposed, level-major column order)
    h^T/c^T per level [512=4x128, cols]  col = tree*2^l + node
  With tree-major column order, the children of parent column j are child
  columns 2j, 2j+1 of the level below -- child-sum and f*c reductions are
  stride-2 vector ops, and each parent chunk [p0,p0+P) consumes exactly
  child columns [2p0, 2p0+2P).

  Per level, per 512-column chunk:
    iou^T accumulated in PSUM: W_ioux^T x^T (3 K-chunks) + W_iouh^T hsum^T
    (4 K-chunks), evacuated by ACT with fused bias+sigmoid/tanh.
    f^T for both children in one go over child columns: W_fh^T h_child^T
    + W_fx^T x_dup^T (parent x duplicated via 0-step AP), sigmoid evac.
    c = sig(i)*tanh(u) + f_even*c_even + f_odd*c_odd ; h = sig(o)*tanh(c).
  h,c level state is bounced through DRAM scratch (SBUF can't hold the
  leaf levels); chunk-granular dependencies let Tile pipeline levels.

  Matmuls run as float32r (full-rate fp32, ~1e-4 rel err on TRN2).
"""

import sys

sys.path.insert(0, "/opt/trn_rl_repo")

from contextlib import ExitStack

import numpy as np

import concourse.bass as bass
import concourse.tile as tile
from concourse import bacc, mybir

F32 = mybir.dt.float32
F32R = mybir.dt.float32r
BF16 = mybir.dt.bfloat16
AFT = mybir.ActivationFunctionType

B, NTREE, DIN, H = 256, 511, 300, 512
NCORES = 8
BC = B // NCORES  # 32 trees per core
DEPTH = 9
KX = 3  # K chunks for D_IN (300 -> 3*128 padded)
KH = 4  # K chunks for H (512 = 4*128)
LCOLS = [BC * (1 << l) for l in range(DEPTH)]  # cols per level, index=level
TOTCOLS = sum(LCOLS)  # 16352
# column offset of each level in x^T (level-major, descending level)
LOFF = {}
_off = 0
for _l in range(DEPTH - 1, -1, -1):
    LOFF[_l] = _off
    _off += LCOLS[_l]

CHUNK = 512
# chunk-major layout tables: per level (desc), chunk width + number of chunks
PW = {l: min(CHUNK, LCOLS[l]) for l in range(DEPTH)}
NCH = {l: (LCOLS[l] + PW[l] - 1) // PW[l] for l in range(DEPTH)}
# global chunk index base per level for the x^T slab
CIBASE = {}
_ci = 0
for _l in range(DEPTH - 1, -1, -1):
    CIBASE[_l] = _ci
    _ci += NCH[_l]
NCHTOT = _ci  # 35


def build_program():
    nc = bacc.Bacc("TRN2", target_bir_lowering=False, debug=False)

    d_xt = nc.dram_tensor("xt", [NCHTOT, 128, KX * CHUNK], BF16, kind="ExternalInput").ap()
    d_wioux = nc.dram_tensor("wioux", [KX, 128, 3 * H], BF16, kind="ExternalInput").ap()
    d_wiouh = nc.dram_tensor("wiouh", [KH, 128, 3 * H], BF16, kind="ExternalInput").ap()
    d_wfx = nc.dram_tensor("wfx", [KX, 128, H], BF16, kind="ExternalInput").ap()
    d_wfh = nc.dram_tensor("wfh", [KH, 128, H], BF16, kind="ExternalInput").ap()
    d_biou = nc.dram_tensor("biou", [128, 12], F32, kind="ExternalInput").ap()
    d_bf = nc.dram_tensor("bf", [128, 4], F32, kind="ExternalInput").ap()

    d_cout = nc.dram_tensor("c_out", [128, 4, BC], F32, kind="ExternalOutput").ap()
    d_hout = nc.dram_tensor("h_out", [128, 4, BC], F32, kind="ExternalOutput").ap()

    with tile.TileContext(nc) as tc, ExitStack() as ctx:
        wpool = ctx.enter_context(tc.tile_pool(name="weights", bufs=1))
        xpool = ctx.enter_context(tc.tile_pool(name="x", bufs=4))
        hchp = ctx.enter_context(tc.tile_pool(name="hch", bufs=4))
        cchp = ctx.enter_context(tc.tile_pool(name="cch", bufs=4))
        outp = ctx.enter_context(tc.tile_pool(name="state", bufs=4))
        workp = ctx.enter_context(tc.tile_pool(name="work", bufs=3))
        fcp = ctx.enter_context(tc.tile_pool(name="fc", bufs=2))
        hsump = ctx.enter_context(tc.tile_pool(name="hsum", bufs=3))
        psump = ctx.enter_context(tc.tile_pool(name="psum", bufs=8, space="PSUM"))
        dramp = ctx.enter_context(tc.tile_pool(name="dram", bufs=1, space="DRAM"))

        # ---- weights / biases (one-time casting DMAs to f32r) ----
        s_wioux = wpool.tile([128, KX, 3 * H], BF16)
        s_wiouh = wpool.tile([128, KH, 3 * H], BF16)
        s_wfx = wpool.tile([128, KX, H], BF16)
        s_wfh = wpool.tile([128, KH, H], BF16)
        for k in range(KX):
            nc.sync.dma_start(out=s_wioux[:, k, :], in_=d_wioux[k])
            nc.sync.dma_start(out=s_wfx[:, k, :], in_=d_wfx[k])
        for k in range(KH):
            nc.sync.dma_start(out=s_wiouh[:, k, :], in_=d_wiouh[k])
            nc.sync.dma_start(out=s_wfh[:, k, :], in_=d_wfh[k])
        s_biou = wpool.tile([128, 12], F32)
        s_bf = wpool.tile([128, 4], F32)
        nc.sync.dma_start(out=s_biou, in_=d_biou)
        nc.sync.dma_start(out=s_bf, in_=d_bf)

        # ---- DRAM scratch for per-level h/c state (levels 8..1) ----
        hd = {}
        cd = {}
        for l in range(1, DEPTH):
            if NCH[l] <= 2:
                continue  # small levels stay SBUF-resident
            hd[l] = dramp.tile([NCH[l], 128, KH, PW[l]], BF16, tag=f"hd{l}", name=f"hd{l}")
            cd[l] = dramp.tile([NCH[l], 128, KH, PW[l]], F32, tag=f"cd{l}", name=f"cd{l}")

        def dup_ap(base):
            """Each column of `base` twice: [128, W] -> [128, W, 2] (0-step)."""
            return bass.AP(
                tensor=base.tensor,
                offset=base.offset,
                ap=list(base.ap) + [[0, 2]],
            )

        def iou_psum(m, P, xt, hsum):
            """PSUM accumulation for iou feature chunk m over P cols."""
            ps = psump.tile([128, CHUNK], F32, tag="ps")
            last_x = hsum is None
            for k in range(KX):
                nc.tensor.matmul(
                    ps[:, :P],
                    s_wioux[:, k, 128 * m : 128 * m + 128],
                    xt[:, k, :P],
                    start=(k == 0),
                    stop=(last_x and k == KX - 1),
                )
            if hsum is not None:
                for k in range(KH):
                    nc.tensor.matmul(
                        ps[:, :P],
                        s_wiouh[:, k, 128 * m : 128 * m + 128],
                        hsum[:, k, :P],
                        start=False,
                        stop=(k == KH - 1),
                    )
            return ps

        sbuf_child = {}  # level -> (list of (ht, ct) per chunk, chunk width)

        def process_level(l):
            C = LCOLS[l]
            P = min(CHUNK, C)
            is_leaf = l == DEPTH - 1
            for j in range(0, C, P):
                # x^T chunk [128, KX, 512] -- one contiguous chunk-major load
                ci = CIBASE[l] + j // P
                xt = xpool.tile([128, KX, CHUNK], BF16, tag="xt")
                nc.sync.dma_start(
                    out=xt[:].rearrange("p k c -> p (k c)"), in_=d_xt[ci]
                )

                ct = outp.tile([128, KH, CHUNK], F32, tag="ct")
                ht = outp.tile([128, KH, CHUNK], BF16, tag="ht")

                hsum = None
                if not is_leaf:
                    # children: cols [2j, 2j+2P) of level l+1 = child chunks
                    if l + 1 in sbuf_child:
                        tiles, pw = sbuf_child[l + 1]
                        npieces = 2 * P // pw
                        cj0 = 2 * j // pw
                        hch = [tiles[cj0 + pc][0] for pc in range(npieces)]
                        cch = [tiles[cj0 + pc][1] for pc in range(npieces)]
                    else:
                        pw = PW[l + 1]
                        npieces = 2 * P // pw
                        cj0 = 2 * j // pw
                        hch, cch = [], []
                        for pc in range(npieces):
                            hc = hchp.tile([128, KH, CHUNK], BF16, tag="hch")
                            cc = cchp.tile([128, KH, CHUNK], F32, tag="cch")
                            nc.sync.dma_start(out=hc[:, :, :pw], in_=hd[l + 1][cj0 + pc])
                            nc.sync.dma_start(out=cc[:, :, :pw], in_=cd[l + 1][cj0 + pc])
                            hch.append(hc)
                            cch.append(cc)

                    # hsum[:, :, a:a+pw/2] = hch[...,0::2] + [...,1::2]
                    hsum = hsump.tile([128, KH, CHUNK], BF16, tag="hsum")
                    for pc in range(npieces):
                        pair = hch[pc][:, :, :pw].rearrange(
                            "p k (n two) -> p k n two", two=2
                        )
                        a = pc * (pw // 2)
                        nc.gpsimd.tensor_add(
                            out=hsum[:, :, a : a + pw // 2],
                            in0=pair[:, :, :, 0],
                            in1=pair[:, :, :, 1],
                        )

                # ---- i/u gates: c = sigmoid(i) * tanh(u) ----
                for m in range(4):
                    ps_u = iou_psum(8 + m, P, xt, hsum)
                    tu = workp.tile([128, CHUNK], F32, tag="tu")
                    nc.scalar.activation(
                        tu[:, :P], ps_u[:, :P], AFT.Tanh, bias=s_biou[:, 8 + m : 9 + m]
                    )
                    ps_i = iou_psum(m, P, xt, hsum)
                    nc.scalar.activation(
                        ct[:, m, :P], ps_i[:, :P], AFT.Sigmoid, bias=s_biou[:, m : m + 1]
                    )
                    nc.vector.tensor_mul(ct[:, m, :P], ct[:, m, :P], tu[:, :P])

                # ---- forget gates + fc accumulation into c ----
                if not is_leaf:
                    for m in range(4):
                        # fx for parent cols, with the forget bias folded in
                        ps_fx = psump.tile([128, CHUNK], F32, tag="ps")
                        for k in range(KX):
                            nc.tensor.matmul(
                                ps_fx[:, :P],
                                s_wfx[:, k, 128 * m : 128 * m + 128],
                                xt[:, k, :P],
                                start=(k == 0),
                                stop=(k == KX - 1),
                            )
                        fx = workp.tile([128, CHUNK], F32, tag="fx")
                        nc.scalar.activation(
                            fx[:, :P], ps_fx[:, :P], AFT.Identity, scale=1.0,
                            bias=s_bf[:, m : m + 1],
                        )
                        fc = fcp.tile([128, 2 * CHUNK], F32, tag="fc")
                        for pc in range(npieces):
                            s = pc * pw
                            ps = psump.tile([128, CHUNK], F32, tag="ps")
                            for k in range(KH):
                                nc.tensor.matmul(
                                    ps[:, :pw],
                                    s_wfh[:, k, 128 * m : 128 * m + 128],
                                    hch[pc][:, k, :pw],
                                    start=(k == 0),
                                    stop=(k == KH - 1),
                                )
                            # f_pre = fh_psum + fx(parent, duplicated via 0-step)
                            f = workp.tile([128, CHUNK], F32, tag="f")
                            nc.vector.scalar_tensor_tensor(
                                out=f[:, :pw],
                                in0=ps[:, :pw],
                                scalar=1.0,
                                in1=dup_ap(fx[:, s // 2 : s // 2 + pw // 2]),
                                op0=mybir.AluOpType.mult,
                                op1=mybir.AluOpType.add,
                            )
                            nc.scalar.activation(f[:, :pw], f[:, :pw], AFT.Sigmoid)
                            nc.vector.tensor_mul(
                                fc[:, s : s + pw], f[:, :pw], cch[pc][:, m, :pw]
                            )
                        fpair = fc[:, : 2 * P].rearrange("p (n two) -> p n two", two=2)
                        nc.gpsimd.tensor_add(
                            out=ct[:, m, :P], in0=ct[:, m, :P], in1=fpair[:, :, 0]
                        )
                        nc.gpsimd.tensor_add(
                            out=ct[:, m, :P], in0=ct[:, m, :P], in1=fpair[:, :, 1]
                        )

                # ---- o gates, h = sigmoid(o) * tanh(c) ----
                for m in range(4):
                    ps_o = iou_psum(4 + m, P, xt, hsum)
                    so = workp.tile([128, CHUNK], F32, tag="so")
                    nc.scalar.activation(
                        so[:, :P], ps_o[:, :P], AFT.Sigmoid, bias=s_biou[:, 4 + m : 5 + m]
                    )
                    tc_ = workp.tile([128, CHUNK], F32, tag="tc")
                    nc.scalar.activation(tc_[:, :P], ct[:, m, :P], AFT.Tanh)
                    nc.vector.tensor_mul(ht[:, m, :P], so[:, :P], tc_[:, :P])

                # ---- store state ----
                if l == 0:
                    nc.gpsimd.dma_start(out=d_hout, in_=ht[:, :, :P])
                    nc.sync.dma_start(out=d_cout, in_=ct[:, :, :P])
                elif NCH[l] <= 2:
                    # small level: children consumed straight from SBUF
                    sbuf_child.setdefault(l, ([], P))[0].append((ht, ct))
                else:
                    nc.sync.dma_start(out=hd[l][j // P], in_=ht[:, :, :P])
                    nc.sync.dma_start(out=cd[l][j // P], in_=ct[:, :, :P])

        for l in range(DEPTH - 1, -1, -1):
            process_level(l)

    nc.compile()
    return nc


_nc_cache = None


def get_program():
    global _nc_cache
    if _nc_cache is None:
        _nc_cache = build_program()
    return _nc_cache


def prep_inputs(inputs, W_ioux, b_ioux, W_iouh, b_iouh, W_fx, b_fx, W_fh, b_fh):
    """Host-side prep: per-core x^T slabs + padded weight chunks + fused biases."""
    inputs = np.ascontiguousarray(np.asarray(inputs, dtype=np.float32))

    import ml_dtypes

    BF = ml_dtypes.bfloat16

    def padk(w, rows):
        w = np.asarray(w, np.float32)
        out = np.zeros((rows * 128, w.shape[1]), np.float32)
        out[: w.shape[0]] = w
        return np.ascontiguousarray(out.reshape(rows, 128, w.shape[1]).astype(BF))

    wioux = padk(W_ioux, KX)
    wiouh = padk(W_iouh, KH)
    wfx = padk(W_fx, KX)
    wfh = padk(W_fh, KH)
    biou = np.ascontiguousarray(
        (np.asarray(b_ioux) + np.asarray(b_iouh)).astype(np.float32).reshape(12, 128).T
    )
    bf = np.ascontiguousarray(
        (np.asarray(b_fx) + np.asarray(b_fh)).astype(np.float32).reshape(4, 128).T
    )

    in_maps = []
    for c in range(NCORES):
        xc = inputs[c * BC : (c + 1) * BC]  # [BC, NTREE, DIN]
        blocks = []
        for l in range(DEPTH - 1, -1, -1):
            a, b = (1 << l) - 1, (1 << (l + 1)) - 1
            blocks.append(xc[:, a:b, :].reshape(-1, DIN))  # tree-major
        xcols = np.concatenate(blocks, axis=0)  # [TOTCOLS, DIN]
        xtf = np.zeros((KX * 128, TOTCOLS), np.float32)
        xtf[:DIN] = xcols.T
        xtf = xtf.reshape(KX, 128, TOTCOLS)
        xt = np.zeros((NCHTOT, 128, KX, CHUNK), np.float32)
        for l in range(DEPTH - 1, -1, -1):
            for jj in range(NCH[l]):
                a = LOFF[l] + jj * PW[l]
                xt[CIBASE[l] + jj, :, :, : PW[l]] = xtf[:, :, a : a + PW[l]].transpose(
                    1, 0, 2
                )
        xt = np.ascontiguousarray(xt.reshape(NCHTOT, 128, KX * CHUNK).astype(BF))
        in_maps.append(
            {
                "xt": xt,
                "wioux": wioux,
                "wiouh": wiouh,
                "wfx": wfx,
                "wfh": wfh,
                "biou": biou,
                "bf": bf,
            }
        )
    return in_maps


def assemble_output(results):
    """results: list of per-core dicts with c_out/h_out [128, 4, BC]."""
    cs, hs = [], []
    for r in results:
        # [128part=feat%128, m=feat//128, tree] -> [tree, 512]
        c = np.transpose(r["c_out"], (2, 1, 0)).reshape(BC, H)
        h = np.transpose(r["h_out"], (2, 1, 0)).reshape(BC, H)
        cs.append(c)
        hs.append(h)
    return np.concatenate(cs, 0), np.concatenate(hs, 0)


def run_on_hw(in_maps, trace=False, tmpdir=None):
    from concourse.bass_utils import run_bass_kernel_spmd

    nc = get_program()
    return run_bass_kernel_spmd(
        nc, in_maps, list(range(NCORES)), trace=trace, tmpdir=tmpdir
    )


def kernel(**inputs):
    in_maps = prep_inputs(**inputs)
    res = run_on_hw(in_maps)
    return assemble_output(res.results)
